# revision 43
# baseline (speedup 1.0000x reference)
"""Trainium2 Bass kernel for nn_Memory_45964740002665 (scatter_memory).

Distribution (8 NeuronCores):
  * Update phase sharded by memory-slot block: core c owns slots
    [576c, 576c+576). It computes score[8192 queries, 576 slots] with
    f32r matmuls from a host-prenormalized qT (4KB/partition staged
    loads), psum->sbuf copies on the scalar engine (DVE sliver for
    balance), per-query row-max via batched DVE reduces, and per-slot
    column-max via strided DVE batch reduces scheduled into the
    AllReduce shadow and phase-2 slack.
  * One 32KB AllReduce(max) of the row-max partials; the one-time
    ~35us collective barrier overlaps phase 1.
  * qu^T = qs^T @ onehot as bf16 matmuls (selection already decided in
    fp32 via is_ge against the all-reduced row max; exp factors folded
    into per-partition scales).
  * Each core normalizes its mem block (rsqrt via Ln/Exp + one Newton
    step, batched across slot tiles), computes G_c = mem_c^T@mem_c in
    bf16, AllGathers the 8 G_k (1MB bf16) while a PE warm pass against
    the local G hides the collective latency.
  * Read phase sharded by embedding column: out[e,nq_shard,k,:] =
    emb_e[:,nq_shard]^T @ G_k in bf16, written as bf16 with 4KB
    contiguous per-partition lines; host transposes k<->nq and upcasts.
"""
import os
import sys

sys.path.insert(0, "/opt/trn_rl_repo")

import numpy as np
import ml_dtypes
import concourse.bacc as bacc
import concourse.mybir as mybir
import concourse.tile as tile
from concourse import masks
from concourse.bass_utils import run_bass_kernel_spmd

F32 = mybir.dt.float32
F32R = mybir.dt.float32r
BF16 = mybir.dt.bfloat16
ALU = mybir.AluOpType
AF = mybir.ActivationFunctionType

N, D, M_SLOTS, C, NQ = 8192, 256, 4608, 9, 16384
NCORES = 8
BLK = M_SLOTS // NCORES          # 576 slots per core
SHARD = NQ // NCORES             # 2048 embedding columns per core
NI = N // 128                    # 64 query chunks
NIC = 8                          # qT staging chunk (ics per chunk)
JCH = [(0, 288), (288, 288)]     # j-chunks within the block (each own psum bank)
NT = (BLK + 127) // 128          # 5 slot tiles (4x128 + 64)
EPS = 1e-12

_CACHE = {}


def _build():
    no_coll = os.environ.get("KNL_NO_COLL", "0") == "1"
    p1_f32 = os.environ.get("KNL_P1_F32", "0") == "1"
    nc = bacc.Bacc("TRN2", target_bir_lowering=False, debug=False,
                   num_devices=NCORES)
    RG = [list(range(NCORES))]
    QDT = F32 if p1_f32 else F32R

    QNT = nc.dram_tensor("qnT", [D, N], QDT, kind="ExternalInput").ap()
    QBF = nc.dram_tensor("qbf", [128, NI, D], BF16, kind="ExternalInput").ap()
    KB = nc.dram_tensor("kb", [BLK, D], F32, kind="ExternalInput").ap()
    EMB = nc.dram_tensor("emb", [2, D, SHARD], BF16, kind="ExternalInput").ap()
    SCAL = nc.dram_tensor("scal", [128, NT, 3], F32, kind="ExternalInput").ap()
    OUT = nc.dram_tensor("out", [2, SHARD, NCORES * D], BF16,
                         kind="ExternalOutput").ap()

    with tile.TileContext(nc) as tc:
        with (
            tc.tile_pool(name="const", bufs=1) as cp,
            tc.tile_pool(name="dram", bufs=1, space="DRAM") as dp,
        ):
            ident = cp.tile([128, 128], F32, tag="ident")
            masks.make_identity(nc, ident[:])

            scal = cp.tile([128, NT, 3], F32, tag="scal")
            nc.sync.dma_start(scal[:], SCAL[:])

            # ---- keys block -> keysT [2][128, BLK] (d-major, f32r) ----
            keysT = [cp.tile([128, BLK], QDT, tag=f"keysT{h}",
                             name=f"keysT{h}") for h in range(2)]
            keysN = [cp.tile([128, D], F32, tag=f"keysN{t}",
                             name=f"keysN{t}") for t in range(NT)]
            with tc.tile_pool(name="pt0", bufs=2, space="PSUM") as pt0:
                for t in range(NT):
                    w = min(128, BLK - t * 128)
                    nc.sync.dma_start(keysN[t][0:w, :],
                                      KB[t * 128:t * 128 + w, :])
                    for h in range(2):
                        ps = pt0.tile([128, 128], F32, tag="pt")
                        nc.tensor.transpose(ps[0:128, 0:w],
                                            keysN[t][0:w, h * 128:(h + 1) * 128],
                                            ident[0:w, 0:w])
                        nc.scalar.copy(keysT[h][:, t * 128:t * 128 + w],
                                       ps[0:128, 0:w])

            rmaxP = cp.tile([128, NI], F32, tag="rmaxP")
            colmaxP = cp.tile([128, BLK], F32, tag="colmaxP")
            rmaxG = cp.tile([128, NI], F32, tag="rmaxG")
            expg = cp.tile([128, NI], F32, tag="expg")

            # collective buffers (single row-max AllReduce)
            ar_in = dp.tile([128, NI], F32, tag="ar_in")
            ar_out = dp.tile([128, NI], F32, tag="ar_out",
                             addr_space=("Local" if no_coll else "Shared"))
            g_in = dp.tile([D, D], BF16, tag="g_in")
            g_out = dp.tile([NCORES, D, D], BF16, tag="g_out",
                            addr_space=("Local" if no_coll else "Shared"))

            with tc.tile_pool(name="score", bufs=1) as scp:
                score = scp.tile([128, NI, BLK], F32, tag="score")

                # ============ phase 1: score = qn @ kb^T ==========
                with tc.tile_pool(name="p2ps", bufs=1, space="PSUM") as pp2:
                    pqu = [[pp2.tile([128, 288], F32, tag=f"pqu{h}{j}",
                                     name=f"pqu{h}{j}")
                            for j in range(2)] for h in range(2)]

                    def p1_chunk(p1q, pp1, c0):
                        # staged load of 8 ic columns of qnT (4KB/partition)
                        qst = p1q.tile([128, 2, NIC * 128], QDT, tag="qst")
                        for h in range(2):
                            nc.sync.dma_start(
                                qst[:, h, :],
                                QNT[h * 128:(h + 1) * 128,
                                    c0 * 128:c0 * 128 + NIC * 128])
                        for ic in range(c0, c0 + NIC):
                            o = (ic - c0) * 128
                            # same stationary tile for consecutive matmuls
                            psj = [pp1.tile([128, 288], F32, tag=f"ps{j}",
                                            name=f"ps{j}_{ic}")
                                   for j in range(2)]
                            for h in range(2):
                                for j, (j0, jw) in enumerate(JCH):
                                    nc.tensor.matmul(psj[j][:, 0:jw],
                                                     qst[:, h, o:o + 128],
                                                     keysT[h][:, j0:j0 + jw],
                                                     start=(h == 0),
                                                     stop=(h == 1))
                            for j, (j0, jw) in enumerate(JCH):
                                # psum->sbuf copy: scalar engine, with a
                                # sliver on DVE to balance the errata cost
                                if j == 1 and ic % 4 == 3:
                                    nc.vector.tensor_copy(
                                        score[:, ic, j0:j0 + jw],
                                        psj[j][:, 0:jw])
                                else:
                                    nc.scalar.copy(score[:, ic, j0:j0 + jw],
                                                   psj[j][:, 0:jw])
                        # batched row-max over the 8 fresh ic rows
                        nc.vector.reduce_max(
                            rmaxP[:, c0:c0 + NIC],
                            score[:, c0:c0 + NIC, :],
                            axis=mybir.AxisListType.X)

                    def colmax_batch(c0):
                        # batched column-max partial (strided over the ic axis)
                        if c0 == 0:
                            nc.vector.reduce_max(
                                colmaxP[:],
                                score[:, c0:c0 + NIC, :]
                                .rearrange("p i b -> p b i"),
                                axis=mybir.AxisListType.X)
                        else:
                            cmb = cp.tile([128, BLK], F32, tag="cmb")
                            nc.vector.reduce_max(
                                cmb[:],
                                score[:, c0:c0 + NIC, :]
                                .rearrange("p i b -> p b i"),
                                axis=mybir.AxisListType.X)
                            nc.vector.tensor_tensor(
                                out=colmaxP[:], in0=cmb[:],
                                in1=colmaxP[:], op=ALU.max)

                    def ar_issue():
                        nc.sync.dma_start(ar_in[:], rmaxP[:])
                        if no_coll:
                            nc.sync.dma_start(ar_out.opt(), ar_in.opt())
                        else:
                            nc.gpsimd.collective_compute(
                                "AllReduce", ALU.max, replica_groups=RG,
                                ins=[ar_in.opt()], outs=[ar_out.opt()])

                    def ar_land():
                        nc.sync.dma_start(rmaxG[:], ar_out[:])
                        nc.scalar.activation(expg[:], rmaxG[:], AF.Exp)

                    with (
                        tc.tile_pool(name="p1q", bufs=2) as p1q,
                        tc.tile_pool(name="p1ps", bufs=2, space="PSUM") as pp1,
                    ):
                        for c0 in range(0, NI, NIC):
                            p1_chunk(p1q, pp1, c0)
                        ar_issue()
                        # colmax batches 0-3 fill the DVE while the AR flies
                        for c0 in range(0, NI // 2, NIC):
                            colmax_batch(c0)

                    # prefetch embeddings while collectives fly
                    embr = [[cp.tile([128, SHARD], BF16, tag=f"embr{e}{h}",
                                     name=f"embr{e}{h}")
                             for h in range(2)] for e in range(2)]
                    for e in range(2):
                        for h in range(2):
                            nc.sync.dma_start(
                                embr[e][h][:],
                                EMB[e, h * 128:(h + 1) * 128, :])

                    ar_land()

                    # ========= phase 2: qu^T = qs^T @ onehot (bf16) =========
                    with (
                        tc.tile_pool(name="p2sb", bufs=4) as p2,
                        tc.tile_pool(name="p2qb", bufs=2) as p2q,
                    ):
                        qbs = [None]
                        QBC = 16   # ics per staged qbf load (8KB/partition)

                        def p2_ic(ic):
                            if ic % QBC == 0:
                                qbs[0] = p2q.tile([128, QBC, D], BF16,
                                                  tag="qbs",
                                                  name=f"qbs{ic}")
                                nc.sync.dma_start(qbs[0][:],
                                                  QBF[:, ic:ic + QBC, :])
                            qs = p2.tile([128, D], BF16, tag="qs")
                            nc.scalar.activation(qs[:], qbs[0][:, ic % QBC, :],
                                                 AF.Copy,
                                                 scale=expg[:, ic:ic + 1])
                            m0 = p2.tile([128, BLK], BF16, tag="m0")
                            nc.vector.tensor_scalar(
                                out=m0[:], in0=score[:, ic, :],
                                scalar1=rmaxG[:, ic:ic + 1], scalar2=None,
                                op0=ALU.is_ge)
                            for h in range(2):
                                for j, (j0, jw) in enumerate(JCH):
                                    nc.tensor.matmul(
                                        pqu[h][j][:, 0:jw],
                                        qs[:, h * 128:(h + 1) * 128],
                                        m0[:, j0:j0 + jw],
                                        start=(ic == 0), stop=(ic == NI - 1))

                        for ic in range(NI):
                            p2_ic(ic)
                            if ic in (8, 24, 40, 56):
                                colmax_batch((4 + (8, 24, 40, 56).index(ic))
                                             * NIC)

                    # ---- colmax finalize: [128, BLK] -> cm [128, NT] ----
                    cmP = cp.tile([128, NT], F32, tag="cmP")
                    with tc.tile_pool(name="ptc", bufs=2, space="PSUM") as ptc:
                        for t in range(NT):
                            w = min(128, BLK - t * 128)
                            ps = ptc.tile([128, 128], F32, tag="ptc")
                            nc.tensor.transpose(ps[0:w, :],
                                                colmaxP[:, t * 128:t * 128 + w],
                                                ident[:])
                            nc.vector.reduce_max(cmP[0:w, t:t + 1], ps[0:w, :],
                                                 axis=mybir.AxisListType.X)
                    # pre = active * exp(-colmax)
                    emcm = cp.tile([128, NT], F32, tag="emcm")
                    nc.scalar.activation(emcm[:], cmP[:], AF.Exp, scale=-1.0)
                    preNT = cp.tile([128, NT], F32, tag="preNT")
                    nc.vector.scalar_tensor_tensor(
                        out=preNT[:], in0=scal[:, :, 2], scalar=0.0,
                        in1=emcm[:], op0=ALU.bypass, op1=ALU.mult)

                    # ========= phase 3: mem block + G_c, AllGather G =========
                    memt = [cp.tile([128, D], BF16, tag=f"memt{t}",
                                    name=f"memt{t}") for t in range(NT)]
                    gsb = [cp.tile([128, D], BF16, tag=f"gsb{h}",
                                   name=f"gsb{h}") for h in range(2)]
                    with (
                        tc.tile_pool(name="p3sb", bufs=2) as p3,
                        tc.tile_pool(name="p3u", bufs=1) as p3u,
                        tc.tile_pool(name="p3ps", bufs=2, space="PSUM") as pp3,
                        tc.tile_pool(name="p3pg", bufs=1, space="PSUM") as pp3g,
                    ):
                        uts = [p3u.tile([128, D], F32, tag=f"u{t}",
                                        name=f"u{t}") for t in range(NT)]
                        ss = p3u.tile([128, NT], F32, tag="ss")
                        # stage qu^T from psum to sbuf (transpose needs SBUF src)
                        quTs = [p3u.tile([128, BLK], F32, tag=f"quTs{h}",
                                         name=f"quTs{h}") for h in range(2)]
                        for h in range(2):
                            for j, (j0, jw) in enumerate(JCH):
                                nc.scalar.copy(quTs[h][:, j0:j0 + jw],
                                               pqu[h][j][:, 0:jw])
                        for t in range(NT):
                            w = min(128, BLK - t * 128)
                            qun = p3.tile([128, D], F32, tag="qun")
                            for h in range(2):
                                ps = pp3.tile([128, 128], F32, tag="p3t")
                                nc.tensor.transpose(
                                    ps[0:w, :],
                                    quTs[h][:, t * 128:t * 128 + w],
                                    ident[:])
                                nc.scalar.copy(qun[0:w, h * 128:(h + 1) * 128],
                                               ps[0:w, :])
                            qsc = p3.tile([128, D], F32, tag="qsc")
                            nc.vector.tensor_scalar_mul(qsc[0:w, :], qun[0:w, :],
                                                        preNT[0:w, t:t + 1])
                            # u = temp*keys + qsc
                            nc.vector.scalar_tensor_tensor(
                                out=uts[t][0:w, :], in0=keysN[t][0:w, :],
                                scalar=scal[0:w, t, 0:1], in1=qsc[0:w, :],
                                op0=ALU.mult, op1=ALU.add)
                            sq = p3.tile([128, D], F32, tag="sq")
                            nc.scalar.square(sq[0:w, :], uts[t][0:w, :])
                            nc.vector.reduce_sum(ss[0:w, t:t + 1], sq[0:w, :],
                                                 axis=mybir.AxisListType.X)
                        # mem = u / ||u||  (row scale cancels; rsqrt via
                        # ln/exp LUT + one Newton step, batched over tiles)
                        nc.vector.tensor_scalar_max(ss[:], ss[:], 1e-30)
                        lnv = p3u.tile([128, NT], F32, tag="lnv")
                        nc.scalar.activation(lnv[:], ss[:], AF.Ln)
                        y = p3u.tile([128, NT], F32, tag="y")
                        nc.scalar.activation(y[:], lnv[:], AF.Exp, scale=-0.5)
                        y2 = p3u.tile([128, NT], F32, tag="y2")
                        nc.vector.scalar_tensor_tensor(
                            out=y2[:], in0=y[:], scalar=0.0, in1=y[:],
                            op0=ALU.bypass, op1=ALU.mult)
                        sy2 = p3u.tile([128, NT], F32, tag="sy2")
                        nc.vector.scalar_tensor_tensor(
                            out=sy2[:], in0=ss[:], scalar=0.0, in1=y2[:],
                            op0=ALU.bypass, op1=ALU.mult)
                        corr = p3u.tile([128, NT], F32, tag="corr")
                        nc.vector.tensor_scalar(
                            out=corr[:], in0=sy2[:], scalar1=-0.5,
                            scalar2=1.5, op0=ALU.mult, op1=ALU.add)
                        fac = p3u.tile([128, NT], F32, tag="fac")
                        nc.vector.scalar_tensor_tensor(
                            out=fac[:], in0=y[:], scalar=0.0, in1=corr[:],
                            op0=ALU.bypass, op1=ALU.mult)
                        for t in range(NT):
                            w = min(128, BLK - t * 128)
                            nc.vector.tensor_scalar_mul(memt[t][0:w, :],
                                                        uts[t][0:w, :],
                                                        fac[0:w, t:t + 1])

                        pg = [pp3g.tile([128, D], F32, tag=f"pg{h}",
                                        name=f"pg{h}") for h in range(2)]
                        for h in range(2):
                            for t in range(NT):
                                w = min(128, BLK - t * 128)
                                nc.tensor.matmul(
                                    pg[h][:],
                                    memt[t][0:w, h * 128:(h + 1) * 128],
                                    memt[t][0:w, :],
                                    start=(t == 0), stop=(t == NT - 1))
                            nc.scalar.copy(gsb[h][:], pg[h][:])

                    for h in range(2):
                        nc.sync.dma_start(g_in[h * 128:(h + 1) * 128, :],
                                          gsb[h][:])
                    if no_coll:
                        nc.sync.dma_start(g_out[0], g_in[:])
                    else:
                        nc.gpsimd.collective_compute(
                            "AllGather", ALU.bypass, replica_groups=RG,
                            ins=[g_in.opt()], outs=[g_out.opt()])

                    # PE warm pass against the local G while the AllGather
                    # flies (results discarded; keeps the ramped tensor
                    # engine from idling through the collective latency)
                    with tc.tile_pool(name="pwps", bufs=2,
                                      space="PSUM") as ppw:
                        for e in range(2):
                            for q in range(SHARD // 128):
                                psw = ppw.tile([128, D], F32, tag="pw")
                                for h in range(2):
                                    nc.tensor.matmul(
                                        psw[:],
                                        embr[e][h][:, q * 128:(q + 1) * 128],
                                        gsb[h][:],
                                        start=(h == 0), stop=(h == 1))

            # ================= phase 4: reads =========================
            with (
                tc.tile_pool(name="p4sb", bufs=1) as p4c,
                tc.tile_pool(name="p4out", bufs=3) as p4o,
                tc.tile_pool(name="p4ps", bufs=2, space="PSUM") as pp4,
            ):
                gst = [p4c.tile([128, NCORES * D], BF16, tag=f"gst{h}",
                                name=f"gst{h}") for h in range(2)]
                for h in range(2):
                    if no_coll:
                        for k in range(NCORES):
                            nc.sync.dma_start(
                                gst[h][:, k * D:(k + 1) * D],
                                g_out[0, h * 128:(h + 1) * 128, :])
                    else:
                        nc.sync.dma_start(
                            gst[h][:].rearrange("p (k d) -> p k d", k=NCORES),
                            g_out[:, h * 128:(h + 1) * 128, :]
                            .rearrange("k p d -> p k d"))

                for e in range(2):
                    for q in range(SHARD // 128):
                        pso = [pp4.tile([128, 512], F32, tag=f"po{b}",
                                        name=f"po{b}_{e}_{q}")
                               for b in range(4)]
                        for b in range(4):
                            for h in range(2):
                                nc.tensor.matmul(
                                    pso[b][:],
                                    embr[e][h][:, q * 128:(q + 1) * 128],
                                    gst[h][:, b * 512:(b + 1) * 512],
                                    start=(h == 0), stop=(h == 1))
                        ob = p4o.tile([128, NCORES * D], BF16, tag="ob")
                        for b in range(4):
                            if b < 2:
                                nc.scalar.copy(ob[:, b * 512:(b + 1) * 512],
                                               pso[b][:])
                            else:
                                nc.vector.tensor_copy(
                                    ob[:, b * 512:(b + 1) * 512], pso[b][:])
                        nc.sync.dma_start(
                            OUT[e, q * 128:(q + 1) * 128, :], ob[:])

    nc.compile()
    return nc


def _host_prep(query, keys, labels, class_counts):
    query = np.asarray(query, np.float32)
    rnorm = 1.0 / np.maximum(
        np.sqrt((query.astype(np.float32) ** 2).sum(1)), EPS)
    labels = np.asarray(labels)
    part = M_SLOTS // C
    slot_class = np.arange(M_SLOTS) // part
    active = np.isin(slot_class, labels).astype(np.float32)
    last = int(labels.max())
    count = np.float32((labels == last).sum())
    in_part = slot_class == last
    cc = np.asarray(class_counts, np.float32)
    temp = np.where(in_part, cc[last], np.float32(1.0)).astype(np.float32)
    temp2 = (temp + np.where(in_part, count, np.float32(0.0))).astype(np.float32)
    invtemp2 = (np.float32(1.0) / temp2).astype(np.float32)
    qn = (query * rnorm[:, None]).astype(np.float32)
    return qn, active, temp, invtemp2


def kernel(query, embeddings_src, embeddings_tgt, keys, class_counts,
           labels, num_classes, **_ignored):
    if "nc" not in _CACHE:
        _CACHE["nc"] = _build()
    nc = _CACHE["nc"]

    query = np.ascontiguousarray(np.asarray(query, np.float32))
    src = np.ascontiguousarray(np.asarray(embeddings_src, np.float32))
    tgt = np.ascontiguousarray(np.asarray(embeddings_tgt, np.float32))
    keys = np.ascontiguousarray(np.asarray(keys, np.float32))

    qn, active, temp, invtemp2 = _host_prep(query, keys, labels,
                                            class_counts)
    qnT = np.ascontiguousarray(qn.T)
    # bf16 q, pre-swizzled so each SBUF partition line is contiguous:
    # [128 (row-within-chunk), NI, D]
    qbf = np.ascontiguousarray(
        qn.reshape(NI, 128, D).transpose(1, 0, 2)).astype(ml_dtypes.bfloat16)
    embf = np.stack([src, tgt]).astype(ml_dtypes.bfloat16)  # [2, D, NQ]

    in_maps = []
    for c in range(NCORES):
        sl = slice(c * SHARD, (c + 1) * SHARD)
        js = slice(c * BLK, (c + 1) * BLK)
        scal = np.zeros((128, NT, 3), np.float32)
        scal[:, :, 0] = 1.0
        scal[:, :, 1] = 1.0
        for t in range(NT):
            w = min(128, BLK - t * 128)
            j0 = c * BLK + t * 128
            scal[0:w, t, 0] = temp[j0:j0 + w]
            scal[0:w, t, 1] = invtemp2[j0:j0 + w]
            scal[0:w, t, 2] = active[j0:j0 + w]
        in_maps.append({
            "qnT": qnT,
            "qbf": qbf,
            "kb": np.ascontiguousarray(keys[js]),
            "emb": np.ascontiguousarray(embf[:, :, sl]),
            "scal": scal,
        })

    res = run_bass_kernel_spmd(nc, in_maps, list(range(NCORES)),
                               **_CACHE.get("run_kwargs", {}))
    _CACHE["last_result"] = res

    out = np.empty((2, NCORES, NQ, D), np.float32)
    for c in range(NCORES):
        sl = slice(c * SHARD, (c + 1) * SHARD)
        # [2, SHARD, 8*256] bf16 -> [2, 8, SHARD, 256] f32
        oc = np.asarray(res.results[c]["out"]).reshape(2, SHARD, NCORES, D)
        out[:, :, sl, :] = oc.astype(np.float32).transpose(0, 2, 1, 3)
    return out


if __name__ == "__main__":
    import time
    os.environ.setdefault("JAX_PLATFORMS", "cpu")
    sys.path.insert(0, "/root/problem")
    import reference as R

    inputs = {k: (np.asarray(v) if not np.isscalar(v) else v)
              for k, v in R.setup_inputs().items()}
    t0 = time.time()
    got = kernel(**inputs)
    print(f"kernel wall (incl compile): {time.time()-t0:.1f}s")
    exp = np.load("expected.npy")
    scale = np.abs(exp).max()
    err = np.abs(got - exp)
    print("max abs err:", err.max(), " rel-to-absmax:", err.max() / scale)
    print("mean abs err:", err.mean())
